# revision 1
# baseline (speedup 1.0000x reference)
"""Jacobi->Cartesian transform kernel for Trainium2 (8 NeuronCores, SPMD).

Math: for each batch b the reference computes x = inv(A(m_b)) @ r for every
trajectory step, where A is the Cartesian->Jacobi matrix. inv(A) has a closed
form: with M_i = cumsum(m)_i, c_i = m_i / M_i, s_i = c_i * r_i:

    x_k = r_k + s_0 - S_k,   S_k = sum_{i>=k} s_i   (suffix sum over particles)

which holds for all k (including k=0, since c_0 == 1 -> s_0 = r_0).

Device program per (batch, tensor) unit, in the natural [t, (k,d)] layout
(partition = t-block, free = (t_in, k, d)):
    S'[15] = c_15*r[15] - r[0]              (scalar_tensor_tensor, FD=96)
    S'[k]  = c_k *r[k]  + S'[k+1]  k=14..0  (scalar_tensor_tensor, FD=96)
    x      = r - S'                         (tensor_sub, FD=1536)
No transposes, no PE, no PSUM; DMA-bound by design.

Sharding: pure data parallelism, 16 batches per core across 8 cores.
"""

import numpy as np

import concourse.bacc as bacc
import concourse.mybir as mybir
from concourse.tile import TileContext
from concourse.bass_utils import run_bass_kernel_spmd

B, T, N, D = 128, 4096, 16, 3
N_CORES = 8
BPC = B // N_CORES          # batches per core
P = 128                     # partitions
TI = T // P                 # 32 t's per partition
FREE = TI * N * D           # 1536 free elements per partition

_CACHE = {}


def build_bass():
    if "nc" in _CACHE:
        return _CACHE["nc"]
    nc = bacc.Bacc(
        "TRN2",
        target_bir_lowering=False,
        debug=False,
        enable_asserts=False,
        num_devices=N_CORES,
    )
    f32 = mybir.dt.float32
    qj = nc.dram_tensor("qj", [BPC, T, N, D], f32, kind="ExternalInput").ap()
    vj = nc.dram_tensor("vj", [BPC, T, N, D], f32, kind="ExternalInput").ap()
    coef = nc.dram_tensor("coef", [P, BPC * N], f32, kind="ExternalInput").ap()
    q = nc.dram_tensor("q", [BPC, T, N, D], f32, kind="ExternalOutput").ap()
    v = nc.dram_tensor("v", [BPC, T, N, D], f32, kind="ExternalOutput").ap()

    with TileContext(nc) as tc:
        with (
            tc.tile_pool(name="coefp", bufs=1) as coefp,
            tc.tile_pool(name="rp", bufs=3) as rp,
            tc.tile_pool(name="sp", bufs=2) as sp,
        ):
            coef_sb = coefp.tile([P, BPC * N], f32)
            nc.sync.dma_start(out=coef_sb[:], in_=coef)

            for b in range(BPC):
                for src, dst in ((qj, q), (vj, v)):
                    r = rp.tile([P, FREE], f32)
                    nc.sync.dma_start(
                        out=r[:],
                        in_=src[b].rearrange("(p ti) k d -> p (ti k d)", p=P),
                    )
                    r4 = r[:].rearrange("p (ti k d) -> p ti k d", k=N, d=D)
                    s = sp.tile([P, FREE], f32)
                    s4 = s[:].rearrange("p (ti k d) -> p ti k d", k=N, d=D)

                    def ck(k):
                        return coef_sb[:, b * N + k : b * N + k + 1]

                    # S'[15] = c15*r[15] - r[0]
                    nc.vector.scalar_tensor_tensor(
                        out=s4[:, :, N - 1 : N, :],
                        in0=r4[:, :, N - 1 : N, :],
                        scalar=ck(N - 1),
                        in1=r4[:, :, 0:1, :],
                        op0=mybir.AluOpType.mult,
                        op1=mybir.AluOpType.subtract,
                    )
                    # S'[k] = ck*r[k] + S'[k+1]
                    for k in range(N - 2, -1, -1):
                        nc.vector.scalar_tensor_tensor(
                            out=s4[:, :, k : k + 1, :],
                            in0=r4[:, :, k : k + 1, :],
                            scalar=ck(k),
                            in1=s4[:, :, k + 1 : k + 2, :],
                            op0=mybir.AluOpType.mult,
                            op1=mybir.AluOpType.add,
                        )
                    # x = r - S'  (in place into r)
                    nc.vector.tensor_sub(out=r[:], in0=r[:], in1=s[:])
                    nc.sync.dma_start(
                        out=dst[b].rearrange("(p ti) k d -> p (ti k d)", p=P),
                        in_=r[:],
                    )
    nc.compile()
    _CACHE["nc"] = nc
    return nc


def make_in_maps(m, qj, vj):
    M = np.cumsum(m.astype(np.float64), axis=-1)
    c = (m.astype(np.float64) / M).astype(np.float32)  # [B, N]
    in_maps = []
    for core in range(N_CORES):
        bs = slice(core * BPC, (core + 1) * BPC)
        coef_rep = np.ascontiguousarray(
            np.broadcast_to(c[bs].reshape(1, BPC * N), (P, BPC * N))
        )
        in_maps.append(
            {
                "qj": np.ascontiguousarray(qj[bs]),
                "vj": np.ascontiguousarray(vj[bs]),
                "coef": coef_rep,
            }
        )
    return in_maps


def kernel(m, qj, vj):
    nc = build_bass()
    in_maps = make_in_maps(m, qj, vj)
    res = run_bass_kernel_spmd(nc, in_maps, core_ids=list(range(N_CORES)))
    q = np.concatenate([res.results[i]["q"] for i in range(N_CORES)], axis=0)
    v = np.concatenate([res.results[i]["v"] for i in range(N_CORES)], axis=0)
    return q, v


# revision 3
# speedup vs baseline: 611.9835x; 611.9835x over previous
"""Jacobi->Cartesian transform kernel for Trainium2 (8 NeuronCores, SPMD).

Math: for each batch b the reference computes x = inv(A(m_b)) @ r for every
trajectory step, where A is the Cartesian->Jacobi matrix. inv(A) has a closed
form: with M_i = cumsum(m)_i, c_i = m_i / M_i, s_i = c_i * r_i:

    x_k = r_k + s_0 - S_k,   S_k = sum_{i>=k} s_i   (suffix sum over particles)

which holds for all k (including k=0, since c_0 == 1 -> s_0 = r_0).

Device program per (batch, tensor) unit, in the natural [t, (k,d)] layout
(partition = t-block, free = (t_in, k, d)):
    S'[15] = c_15*r[15] - r[0]              (scalar_tensor_tensor, FD=96)
    S'[k]  = c_k *r[k]  + S'[k+1]  k=14..0  (scalar_tensor_tensor, FD=96)
    x      = r - S'                         (tensor_sub, FD=1536)
No transposes, no PE, no PSUM; DMA-bound by design.

Sharding: pure data parallelism, 16 batches per core across 8 cores.
"""

import numpy as np

import concourse.bacc as bacc
import concourse.mybir as mybir
from concourse.tile import TileContext
from concourse.bass_utils import run_bass_kernel_spmd

B, T, N, D = 128, 4096, 16, 3
N_CORES = 8
BPC = B // N_CORES          # batches per core
P = 128                     # partitions
TI = T // P                 # 32 t's per partition
FREE = TI * N * D           # 1536 free elements per partition

_CACHE = {}


def build_bass():
    if "nc" in _CACHE:
        return _CACHE["nc"]
    nc = bacc.Bacc(
        "TRN2",
        target_bir_lowering=False,
        debug=False,
        enable_asserts=False,
        num_devices=N_CORES,
    )
    f32 = mybir.dt.float32
    qj = nc.dram_tensor("qj", [BPC, T, N, D], f32, kind="ExternalInput").ap()
    vj = nc.dram_tensor("vj", [BPC, T, N, D], f32, kind="ExternalInput").ap()
    coef = nc.dram_tensor("coef", [P, BPC * N], f32, kind="ExternalInput").ap()
    q = nc.dram_tensor("q", [BPC, T, N, D], f32, kind="ExternalOutput").ap()
    v = nc.dram_tensor("v", [BPC, T, N, D], f32, kind="ExternalOutput").ap()

    with TileContext(nc) as tc:
        with (
            tc.tile_pool(name="coefp", bufs=1) as coefp,
            tc.tile_pool(name="rp", bufs=6) as rp,
            tc.tile_pool(name="sp", bufs=4) as sp,
        ):
            coef_sb = coefp.tile([P, BPC * N], f32)
            nc.sync.dma_start(out=coef_sb[:], in_=coef)

            for b in range(BPC):
                for src, dst in ((qj, q), (vj, v)):
                    r = rp.tile([P, FREE], f32)
                    nc.sync.dma_start(
                        out=r[:],
                        in_=src[b].rearrange("(p ti) k d -> p (ti k d)", p=P),
                    )
                    r4 = r[:].rearrange("p (ti k d) -> p ti k d", k=N, d=D)
                    s = sp.tile([P, FREE], f32)
                    s4 = s[:].rearrange("p (ti k d) -> p ti k d", k=N, d=D)

                    def ck(k):
                        return coef_sb[:, b * N + k : b * N + k + 1]

                    # S'[15] = c15*r[15] - r[0]
                    nc.vector.scalar_tensor_tensor(
                        out=s4[:, :, N - 1 : N, :],
                        in0=r4[:, :, N - 1 : N, :],
                        scalar=ck(N - 1),
                        in1=r4[:, :, 0:1, :],
                        op0=mybir.AluOpType.mult,
                        op1=mybir.AluOpType.subtract,
                    )
                    # S'[k] = ck*r[k] + S'[k+1]
                    for k in range(N - 2, -1, -1):
                        nc.vector.scalar_tensor_tensor(
                            out=s4[:, :, k : k + 1, :],
                            in0=r4[:, :, k : k + 1, :],
                            scalar=ck(k),
                            in1=s4[:, :, k + 1 : k + 2, :],
                            op0=mybir.AluOpType.mult,
                            op1=mybir.AluOpType.add,
                        )
                    # x = r - S'  (in place into r)
                    nc.vector.tensor_sub(out=r[:], in0=r[:], in1=s[:])
                    nc.sync.dma_start(
                        out=dst[b].rearrange("(p ti) k d -> p (ti k d)", p=P),
                        in_=r[:],
                    )
    nc.compile()
    _CACHE["nc"] = nc
    return nc


def make_in_maps(m, qj, vj):
    m = np.asarray(m, dtype=np.float32)
    qj = np.asarray(qj, dtype=np.float32)
    vj = np.asarray(vj, dtype=np.float32)
    M = np.cumsum(m.astype(np.float64), axis=-1)
    c = (m.astype(np.float64) / M).astype(np.float32)  # [B, N]
    in_maps = []
    for core in range(N_CORES):
        bs = slice(core * BPC, (core + 1) * BPC)
        coef_rep = np.ascontiguousarray(
            np.broadcast_to(c[bs].reshape(1, BPC * N), (P, BPC * N))
        )
        in_maps.append(
            {
                "qj": np.ascontiguousarray(qj[bs]),
                "vj": np.ascontiguousarray(vj[bs]),
                "coef": coef_rep,
            }
        )
    return in_maps


def kernel(m, qj, vj):
    nc = build_bass()
    in_maps = make_in_maps(m, qj, vj)
    res = run_bass_kernel_spmd(nc, in_maps, core_ids=list(range(N_CORES)))
    q = np.concatenate([res.results[i]["q"] for i in range(N_CORES)], axis=0)
    v = np.concatenate([res.results[i]["v"] for i in range(N_CORES)], axis=0)
    return q, v
